# revision 17
# baseline (speedup 1.0000x reference)
"""LIF spiking-neuron scan (SimpleSNN) Trainium2 Bass kernel.

Reference semantics (per sample b, neuron n, over T timesteps):
    mem = mem * 0.9 + x[t]
    spike[t] = (mem >= 1.5)
    mem = mem * (1 - spike[t])

Full inputs [256, 200, 1024] f32 are sharded batch-wise over 8 NeuronCores
(32 samples/core; the time recurrence is per-sample so no cross-core comms).

Host-side, each core's shard [32, 200, 1024] is permuted to a
partition-major layout [128, 200, 256] with partition p = k*32 + b
(k = n // 256, b = sample), so every chunk DMA is a single dense 3-D
transfer carrying one completion semaphore.

Per-core device design (engine pipeline, all stages overlap):
  - DVE (Vector): the recurrence is rewritten over the PRE-reset
    membrane w:
        w_t = select(w_{t-1} < 1.5, w_{t-1}, 0) * 0.9 + x_t
        spike_t = (w_t >= 1.5)
    which is bit-identical to the reference (same two f32 roundings per
    step) and needs only ONE fused custom-DVE op per step. The whole
    sequential chain is 200 back-to-back ~[128, 256] Vector ops at
    ~425 ns each (~85 us — the critical path; a 2-src f32 custom DVE op
    is architecturally limited to 1 elem/cycle @ 0.96 GHz).
  - ACT (Scalar): per 8-step chunk, one sweep emits uint8 spikes:
        spike_u8 = u8(Sigmoid(2^29 * w - 1.5*2^29))
    (1 ulp of w at 1.5 maps to +-64 -> sigmoid fully saturated to exact
    1.0/0.0; only w == 1.5 exactly, measure-zero, could differ). The
    host converts u8 spikes back to f32 (exact) while unsharding.
  - DMA: x loads need ~308 GB/s sustained to feed the DVE chain and the
    16 DMA engines aggregate ~384 GB/s, so continuous spike stores would
    leave zero catch-up margin and every hiccup becomes a Vector stall.
    Loads therefore alternate the SP HWDGE ring / GpSimd SWDGE queue
    with a 12-chunk prefetch cushion, while spike stores are BATCHED:
    sweeps accumulate into a [128, 40, 256] u8 superchunk tile (5 chunks)
    and one store per superchunk (1.3 MB, 5 total) bursts out on the ACT
    ring, mostly after the load stream has drained.
"""

from contextlib import ExitStack

import numpy as np

B, T, N = 256, 200, 1024
NCORES = 8
BL = B // NCORES  # 32 samples per core
DECAY = 0.9
TH = 1.5
P128 = 128
FREE = 256  # free-dim size of the state tile
NK = N // FREE  # 4 n-blocks; partition p = k*32 + b
TC = 8  # steps per chunk
NCH = T // TC  # 25 chunks
SUPER = 5  # chunks per store superchunk
# Sigmoid threshold scale: 1.5 * 2^29 is exactly representable in f32 and
# one f32 ulp of w at 1.5 (1.19e-7) maps to +-64 — deep in sigmoid
# saturation, so the u8 output is an exact (w >= 1.5) indicator.
SIG_SCALE = float(2**29)
SIG_BIAS = -TH * SIG_SCALE

_CACHE = {}

_LIF_OP_NAME = "LIF_STEP_ANT"


def _lif_reference(in0, in1, s0, s1, imm2):
    return (
        np.where(in0 < np.float32(s0), in0, np.float32(0.0)) * np.float32(s1) + in1
    ).astype(np.float32)


def _register_lif_op():
    """Register the fused LIF-step custom DVE op:
        out = select(in0 < s0, in0, 0) * s1 + in1
    (in0 = previous membrane w, in1 = x_t, s0 = threshold, s1 = decay).
    Registration is the runtime equivalent of appending to dve_ops.OPS;
    uops_sha is computed from the same lower() used at compile time.
    """
    import concourse.dve_ops as dve_ops
    from concourse.dve_ops import DveOp
    from concourse.dve_spec import C0, C1, Spec, Src0, Src1, Zero, lower, select
    from concourse.dve_uop import DveOpSpec

    if _LIF_OP_NAME in dve_ops._SUB_OPCODE_FOR_NAME:
        for op in dve_ops.OPS:
            if op.name == _LIF_OP_NAME:
                return op
        raise RuntimeError("LIF op registered but not in OPS")

    body = select(Src0 < C0, Src0, Zero) * C1 + Src1
    spec = Spec(body=body, reference=_lif_reference)
    row = dve_ops._CUSTOM_DVE_ROW_BASE + len(dve_ops.OPS)
    shas = {}
    for ver in ("v3", "v4"):
        uops = lower(spec, ver=ver)
        shas[ver] = DveOpSpec(
            name=_LIF_OP_NAME, opcode=row, uops=uops, rd1_en=True
        ).sha(ver)
    op = DveOp(_LIF_OP_NAME, spec, subdim=False, uops_sha=shas)
    dve_ops.OPS.append(op)
    dve_ops._SUB_OPCODE_FOR_NAME[_LIF_OP_NAME] = row
    dve_ops.CUSTOM_DVE_SPECS[_LIF_OP_NAME] = spec
    return op


def _build_bass(reps: int = 1):
    import concourse.bacc as bacc
    import concourse.tile as tile
    from concourse import mybir

    lif_op = _register_lif_op()

    nc = bacc.Bacc(
        "TRN2",
        target_bir_lowering=False,
        debug=False,
        enable_asserts=False,
    )

    P = P128
    f32 = mybir.dt.float32
    u8 = mybir.dt.uint8

    x_d = nc.dram_tensor("x", [P, T, FREE], f32, kind="ExternalInput").ap()
    s_d = nc.dram_tensor("spk", [P, T, FREE], u8, kind="ExternalOutput").ap()

    with ExitStack() as ctx:
        tc = ctx.enter_context(tile.TileContext(nc))
        xp = ctx.enter_context(tc.tile_pool(name="xp", bufs=12))
        wp = ctx.enter_context(tc.tile_pool(name="wp", bufs=2))
        sp = ctx.enter_context(tc.tile_pool(name="sp", bufs=2))
        st = ctx.enter_context(tc.tile_pool(name="st", bufs=1))

        zero = st.tile([P, FREE], f32)
        nc.vector.memset(zero[:], 0.0)
        sig_bias = st.tile([P, 1], f32)
        nc.vector.memset(sig_bias[:], SIG_BIAS)

        wt_prev = None
        spt = None
        for c in range(NCH * reps):
            t0 = (c % NCH) * TC
            xt = xp.tile([P, TC, FREE], f32, tag="x")
            # Loads alternate between the SP HWDGE ring and the GpSimd
            # SWDGE queue (two independent DMA paths, both otherwise idle)
            # so consecutive chunk loads overlap.
            ld_eng = nc.sync if c % 2 == 0 else nc.gpsimd
            ld_eng.dma_start(out=xt[:], in_=x_d[:, t0 : t0 + TC, :])

            wt = wp.tile([P, TC, FREE], f32, tag="w")
            for j in range(TC):
                if c == 0 and j == 0:
                    w_in = zero[:]
                elif j == 0:
                    w_in = wt_prev[:, TC - 1, :]
                else:
                    w_in = wt[:, j - 1, :]
                # w_t = select(w_{t-1} < TH, w_{t-1}, 0) * DECAY + x_t
                nc.vector._custom_dve(
                    lif_op,
                    out=wt[:, j, :],
                    in0=w_in,
                    in1=xt[:, j, :],
                    s0=TH,
                    s1=DECAY,
                )
            wt_prev = wt

            # spikes = u8(sigmoid(2^29*(w - 1.5))) — exact 0/1 indicator of
            # (w >= 1.5) up to the measure-zero w == 1.5 case — written into
            # this chunk's slice of the superchunk store tile.
            sc = c % SUPER
            if sc == 0:
                spt = sp.tile([P, SUPER * TC, FREE], u8, tag="s")
            nc.scalar.activation(
                out=spt[:, sc * TC : (sc + 1) * TC, :].rearrange(
                    "p t f -> p (t f)"
                ),
                in_=wt[:].rearrange("p t f -> p (t f)"),
                func=mybir.ActivationFunctionType.Sigmoid,
                bias=sig_bias[:],
                scale=SIG_SCALE,
            )
            if sc == SUPER - 1:
                ts = t0 - (SUPER - 1) * TC
                nc.scalar.dma_start(
                    out=s_d[:, ts : ts + SUPER * TC, :], in_=spt[:]
                )

    nc.compile()
    return nc


def _get_nc():
    if "nc" not in _CACHE:
        _CACHE["nc"] = _build_bass()
    return _CACHE["nc"]


def _shard_input(inputs: np.ndarray, i: int) -> np.ndarray:
    # [32, 200, 1024] -> [32, 200, 4, 256] -> [4, 32, 200, 256] -> [128, 200, 256]
    xi = inputs[i * BL : (i + 1) * BL]
    xi = xi.reshape(BL, T, NK, FREE).transpose(2, 0, 1, 3)
    return np.ascontiguousarray(xi).reshape(P128, T, FREE)


def _unshard_output(spk: np.ndarray) -> np.ndarray:
    # u8 [128, 200, 256] -> [4, 32, 200, 256] -> [32, 200, 4, 256]
    # -> [32, 200, 1024] f32 (u8 spikes are exact 0/1)
    s = spk.reshape(NK, BL, T, FREE).transpose(1, 2, 0, 3)
    return np.ascontiguousarray(s).reshape(BL, T, N).astype(np.float32)


def kernel(inputs: np.ndarray, trace: bool = False) -> np.ndarray:
    from concourse.bass_utils import run_bass_kernel_spmd

    inputs = np.ascontiguousarray(np.asarray(inputs, dtype=np.float32))
    assert inputs.shape == (B, T, N), inputs.shape

    nc = _get_nc()
    in_maps = [{"x": _shard_input(inputs, i)} for i in range(NCORES)]
    res = run_bass_kernel_spmd(
        nc, in_maps, core_ids=list(range(NCORES)), trace=trace
    )
    _CACHE["last_results"] = res
    out = np.concatenate(
        [_unshard_output(r["spk"]) for r in res.results], axis=0
    )
    return out


# revision 18
# speedup vs baseline: 1.1742x; 1.1742x over previous
"""LIF spiking-neuron scan (SimpleSNN) Trainium2 Bass kernel.

Reference semantics (per sample b, neuron n, over T timesteps):
    mem = mem * 0.9 + x[t]
    spike[t] = (mem >= 1.5)
    mem = mem * (1 - spike[t])

Full inputs [256, 200, 1024] f32 are sharded batch-wise over 8 NeuronCores
(32 samples/core; the time recurrence is per-sample so no cross-core comms).

Host-side, each core's shard [32, 200, 1024] is permuted to a
partition-major layout [128, 200, 256] with partition p = k*32 + b
(k = n // 256, b = sample), so every chunk DMA is a single dense 3-D
transfer carrying one completion semaphore.

Per-core device design (engine pipeline, all stages overlap):
  - DVE (Vector): the recurrence is rewritten over the PRE-reset
    membrane w:
        w_t = select(w_{t-1} < 1.5, w_{t-1}, 0) * 0.9 + x_t
        spike_t = (w_t >= 1.5)
    which is bit-identical to the reference (same two f32 roundings per
    step) and needs only ONE fused custom-DVE op per step. The whole
    sequential chain is 200 back-to-back ~[128, 256] Vector ops at
    ~425 ns each (~85 us — the critical path; a 2-src f32 custom DVE op
    is architecturally limited to 1 elem/cycle @ 0.96 GHz).
  - ACT (Scalar): per 8-step chunk, one sweep emits uint8 spikes:
        spike_u8 = u8(Sigmoid(2^29 * w - 1.5*2^29))
    (1 ulp of w at 1.5 maps to +-64 -> sigmoid fully saturated to exact
    1.0/0.0; only w == 1.5 exactly, measure-zero, could differ). The
    host converts u8 spikes back to f32 (exact) while unsharding.
  - DMA: x loads need ~308 GB/s sustained to feed the DVE chain and the
    16 DMA engines aggregate ~384 GB/s, so continuous spike stores would
    leave zero catch-up margin and every hiccup becomes a Vector stall.
    Loads therefore alternate the SP HWDGE ring / GpSimd SWDGE queue
    with a 12-chunk prefetch cushion, while spike stores are BATCHED:
    sweeps accumulate into a [128, 40, 256] u8 superchunk tile (5 chunks)
    and one store per superchunk (1.3 MB, 5 total) bursts out on the ACT
    ring, mostly after the load stream has drained.
"""

from contextlib import ExitStack

import numpy as np

B, T, N = 256, 200, 1024
NCORES = 8
BL = B // NCORES  # 32 samples per core
DECAY = 0.9
TH = 1.5
P128 = 128
FREE = 256  # free-dim size of the state tile
NK = N // FREE  # 4 n-blocks; partition p = k*32 + b
TC = 10  # steps per chunk
NCH = T // TC  # 20 chunks
SUPER = 4  # chunks per store superchunk
# Sigmoid threshold scale: 1.5 * 2^29 is exactly representable in f32 and
# one f32 ulp of w at 1.5 (1.19e-7) maps to +-64 — deep in sigmoid
# saturation, so the u8 output is an exact (w >= 1.5) indicator.
SIG_SCALE = float(2**29)
SIG_BIAS = -TH * SIG_SCALE

_CACHE = {}

_LIF_OP_NAME = "LIF_STEP_ANT"


def _lif_reference(in0, in1, s0, s1, imm2):
    return (
        np.where(in0 < np.float32(s0), in0, np.float32(0.0)) * np.float32(s1) + in1
    ).astype(np.float32)


def _register_lif_op():
    """Register the fused LIF-step custom DVE op:
        out = select(in0 < s0, in0, 0) * s1 + in1
    (in0 = previous membrane w, in1 = x_t, s0 = threshold, s1 = decay).
    Registration is the runtime equivalent of appending to dve_ops.OPS;
    uops_sha is computed from the same lower() used at compile time.
    """
    import concourse.dve_ops as dve_ops
    from concourse.dve_ops import DveOp
    from concourse.dve_spec import C0, C1, Spec, Src0, Src1, Zero, lower, select
    from concourse.dve_uop import DveOpSpec

    if _LIF_OP_NAME in dve_ops._SUB_OPCODE_FOR_NAME:
        for op in dve_ops.OPS:
            if op.name == _LIF_OP_NAME:
                return op
        raise RuntimeError("LIF op registered but not in OPS")

    body = select(Src0 < C0, Src0, Zero) * C1 + Src1
    spec = Spec(body=body, reference=_lif_reference)
    row = dve_ops._CUSTOM_DVE_ROW_BASE + len(dve_ops.OPS)
    shas = {}
    for ver in ("v3", "v4"):
        uops = lower(spec, ver=ver)
        shas[ver] = DveOpSpec(
            name=_LIF_OP_NAME, opcode=row, uops=uops, rd1_en=True
        ).sha(ver)
    op = DveOp(_LIF_OP_NAME, spec, subdim=False, uops_sha=shas)
    dve_ops.OPS.append(op)
    dve_ops._SUB_OPCODE_FOR_NAME[_LIF_OP_NAME] = row
    dve_ops.CUSTOM_DVE_SPECS[_LIF_OP_NAME] = spec
    return op


def _build_bass(reps: int = 1):
    import concourse.bacc as bacc
    import concourse.tile as tile
    from concourse import mybir

    lif_op = _register_lif_op()

    nc = bacc.Bacc(
        "TRN2",
        target_bir_lowering=False,
        debug=False,
        enable_asserts=False,
    )

    P = P128
    f32 = mybir.dt.float32
    u8 = mybir.dt.uint8

    x_d = nc.dram_tensor("x", [P, T, FREE], f32, kind="ExternalInput").ap()
    s_d = nc.dram_tensor("spk", [P, T, FREE], u8, kind="ExternalOutput").ap()

    with ExitStack() as ctx:
        tc = ctx.enter_context(tile.TileContext(nc))
        xp = ctx.enter_context(tc.tile_pool(name="xp", bufs=10))
        wp = ctx.enter_context(tc.tile_pool(name="wp", bufs=2))
        sp = ctx.enter_context(tc.tile_pool(name="sp", bufs=2))
        st = ctx.enter_context(tc.tile_pool(name="st", bufs=1))

        zero = st.tile([P, FREE], f32)
        nc.vector.memset(zero[:], 0.0)
        sig_bias = st.tile([P, 1], f32)
        nc.vector.memset(sig_bias[:], SIG_BIAS)

        wt_prev = None
        spt = None
        for c in range(NCH * reps):
            t0 = (c % NCH) * TC
            xt = xp.tile([P, TC, FREE], f32, tag="x")
            # Loads alternate between the SP HWDGE ring and the GpSimd
            # SWDGE queue (two independent DMA paths, both otherwise idle)
            # so consecutive chunk loads overlap.
            ld_eng = nc.sync if c % 2 == 0 else nc.gpsimd
            ld_eng.dma_start(out=xt[:], in_=x_d[:, t0 : t0 + TC, :])

            wt = wp.tile([P, TC, FREE], f32, tag="w")
            for j in range(TC):
                if c == 0 and j == 0:
                    w_in = zero[:]
                elif j == 0:
                    w_in = wt_prev[:, TC - 1, :]
                else:
                    w_in = wt[:, j - 1, :]
                # w_t = select(w_{t-1} < TH, w_{t-1}, 0) * DECAY + x_t
                nc.vector._custom_dve(
                    lif_op,
                    out=wt[:, j, :],
                    in0=w_in,
                    in1=xt[:, j, :],
                    s0=TH,
                    s1=DECAY,
                )
            wt_prev = wt

            # spikes = u8(sigmoid(2^29*(w - 1.5))) — exact 0/1 indicator of
            # (w >= 1.5) up to the measure-zero w == 1.5 case — written into
            # this chunk's slice of the superchunk store tile.
            sc = c % SUPER
            if sc == 0:
                spt = sp.tile([P, SUPER * TC, FREE], u8, tag="s")
            nc.scalar.activation(
                out=spt[:, sc * TC : (sc + 1) * TC, :].rearrange(
                    "p t f -> p (t f)"
                ),
                in_=wt[:].rearrange("p t f -> p (t f)"),
                func=mybir.ActivationFunctionType.Sigmoid,
                bias=sig_bias[:],
                scale=SIG_SCALE,
            )
            if sc == SUPER - 1:
                ts = t0 - (SUPER - 1) * TC
                nc.scalar.dma_start(
                    out=s_d[:, ts : ts + SUPER * TC, :], in_=spt[:]
                )

    nc.compile()
    return nc


def _get_nc():
    if "nc" not in _CACHE:
        _CACHE["nc"] = _build_bass()
    return _CACHE["nc"]


def _shard_input(inputs: np.ndarray, i: int) -> np.ndarray:
    # [32, 200, 1024] -> [32, 200, 4, 256] -> [4, 32, 200, 256] -> [128, 200, 256]
    xi = inputs[i * BL : (i + 1) * BL]
    xi = xi.reshape(BL, T, NK, FREE).transpose(2, 0, 1, 3)
    return np.ascontiguousarray(xi).reshape(P128, T, FREE)


def _unshard_output(spk: np.ndarray) -> np.ndarray:
    # u8 [128, 200, 256] -> [4, 32, 200, 256] -> [32, 200, 4, 256]
    # -> [32, 200, 1024] f32 (u8 spikes are exact 0/1)
    s = spk.reshape(NK, BL, T, FREE).transpose(1, 2, 0, 3)
    return np.ascontiguousarray(s).reshape(BL, T, N).astype(np.float32)


def kernel(inputs: np.ndarray, trace: bool = False) -> np.ndarray:
    from concourse.bass_utils import run_bass_kernel_spmd

    inputs = np.ascontiguousarray(np.asarray(inputs, dtype=np.float32))
    assert inputs.shape == (B, T, N), inputs.shape

    nc = _get_nc()
    in_maps = [{"x": _shard_input(inputs, i)} for i in range(NCORES)]
    res = run_bass_kernel_spmd(
        nc, in_maps, core_ids=list(range(NCORES)), trace=trace
    )
    _CACHE["last_results"] = res
    out = np.concatenate(
        [_unshard_output(r["spk"]) for r in res.results], axis=0
    )
    return out


# revision 20
# speedup vs baseline: 1.2374x; 1.0539x over previous
"""LIF spiking-neuron scan (SimpleSNN) Trainium2 Bass kernel.

Reference semantics (per sample b, neuron n, over T timesteps):
    mem = mem * 0.9 + x[t]
    spike[t] = (mem >= 1.5)
    mem = mem * (1 - spike[t])

Full inputs [256, 200, 1024] f32 are sharded batch-wise over 8 NeuronCores
(32 samples/core; the time recurrence is per-sample so no cross-core comms).

Host-side, each core's shard [32, 200, 1024] is permuted to a
partition-major layout [128, 200, 256] with partition p = k*32 + b
(k = n // 256, b = sample), so every chunk DMA is a single dense 3-D
transfer carrying one completion semaphore.

Per-core device design (engine pipeline, all stages overlap):
  - DVE (Vector): the recurrence is rewritten over the PRE-reset
    membrane w:
        w_t = select(w_{t-1} < 1.5, w_{t-1}, 0) * 0.9 + x_t
        spike_t = (w_t >= 1.5)
    which is bit-identical to the reference (same two f32 roundings per
    step) and needs only ONE fused custom-DVE op per step. The whole
    sequential chain is 200 back-to-back ~[128, 256] Vector ops at
    ~425 ns each (~85 us — the critical path; a 2-src f32 custom DVE op
    is architecturally limited to 1 elem/cycle @ 0.96 GHz).
  - ACT (Scalar): per 8-step chunk, one sweep emits uint8 spikes:
        spike_u8 = u8(Sigmoid(2^29 * w - 1.5*2^29))
    (1 ulp of w at 1.5 maps to +-64 -> sigmoid fully saturated to exact
    1.0/0.0; only w == 1.5 exactly, measure-zero, could differ). The
    host converts u8 spikes back to f32 (exact) while unsharding.
  - DMA: x loads need ~308 GB/s sustained to feed the DVE chain and the
    16 DMA engines aggregate ~384 GB/s, so continuous spike stores would
    leave zero catch-up margin and every hiccup becomes a Vector stall.
    Loads therefore alternate the SP HWDGE ring / GpSimd SWDGE queue
    with a 12-chunk prefetch cushion, while spike stores are BATCHED:
    sweeps accumulate into a [128, 40, 256] u8 superchunk tile (5 chunks)
    and one store per superchunk (1.3 MB, 5 total) bursts out on the ACT
    ring, mostly after the load stream has drained.
"""

from contextlib import ExitStack

import numpy as np

B, T, N = 256, 200, 1024
NCORES = 8
BL = B // NCORES  # 32 samples per core
DECAY = 0.9
TH = 1.5
P128 = 128
FREE = 256  # free-dim size of the state tile
NK = N // FREE  # 4 n-blocks; partition p = k*32 + b
# Ragged chunks: 5-step head chunks (fast ramp — the first LIF op waits
# on a 0.65 MB load only) and 5-step tail chunks (small final sweeps).
CHUNKS = [5, 5] + [10] * 18 + [5, 5]
assert sum(CHUNKS) == T
TCMAX = max(CHUNKS)
# Store groups (superchunks): one spike store per group, sized large in
# the middle (fewer triggers, bursts ride the DMA slack behind the load
# stream) and small at the end (short tail after the chain finishes).
GROUPS = [(0, 6), (6, 10), (10, 14), (14, 18), (18, 20), (20, 21), (21, 22)]
GMAX = max(sum(CHUNKS[a:b]) for a, b in GROUPS)
# Sigmoid threshold scale: 1.5 * 2^29 is exactly representable in f32 and
# one f32 ulp of w at 1.5 (1.19e-7) maps to +-64 — deep in sigmoid
# saturation, so the u8 output is an exact (w >= 1.5) indicator.
SIG_SCALE = float(2**29)
SIG_BIAS = -TH * SIG_SCALE

_CACHE = {}

_LIF_OP_NAME = "LIF_STEP_ANT"


def _lif_reference(in0, in1, s0, s1, imm2):
    return (
        np.where(in0 < np.float32(s0), in0, np.float32(0.0)) * np.float32(s1) + in1
    ).astype(np.float32)


def _register_lif_op():
    """Register the fused LIF-step custom DVE op:
        out = select(in0 < s0, in0, 0) * s1 + in1
    (in0 = previous membrane w, in1 = x_t, s0 = threshold, s1 = decay).
    Registration is the runtime equivalent of appending to dve_ops.OPS;
    uops_sha is computed from the same lower() used at compile time.
    """
    import concourse.dve_ops as dve_ops
    from concourse.dve_ops import DveOp
    from concourse.dve_spec import C0, C1, Spec, Src0, Src1, Zero, lower, select
    from concourse.dve_uop import DveOpSpec

    if _LIF_OP_NAME in dve_ops._SUB_OPCODE_FOR_NAME:
        for op in dve_ops.OPS:
            if op.name == _LIF_OP_NAME:
                return op
        raise RuntimeError("LIF op registered but not in OPS")

    body = select(Src0 < C0, Src0, Zero) * C1 + Src1
    spec = Spec(body=body, reference=_lif_reference)
    row = dve_ops._CUSTOM_DVE_ROW_BASE + len(dve_ops.OPS)
    shas = {}
    for ver in ("v3", "v4"):
        uops = lower(spec, ver=ver)
        shas[ver] = DveOpSpec(
            name=_LIF_OP_NAME, opcode=row, uops=uops, rd1_en=True
        ).sha(ver)
    op = DveOp(_LIF_OP_NAME, spec, subdim=False, uops_sha=shas)
    dve_ops.OPS.append(op)
    dve_ops._SUB_OPCODE_FOR_NAME[_LIF_OP_NAME] = row
    dve_ops.CUSTOM_DVE_SPECS[_LIF_OP_NAME] = spec
    return op


def _build_bass(reps: int = 1):
    import concourse.bacc as bacc
    import concourse.tile as tile
    from concourse import mybir

    lif_op = _register_lif_op()

    nc = bacc.Bacc(
        "TRN2",
        target_bir_lowering=False,
        debug=False,
        enable_asserts=False,
    )

    P = P128
    f32 = mybir.dt.float32
    u8 = mybir.dt.uint8

    x_d = nc.dram_tensor("x", [P, T, FREE], f32, kind="ExternalInput").ap()
    s_d = nc.dram_tensor("spk", [P, T, FREE], u8, kind="ExternalOutput").ap()

    with ExitStack() as ctx:
        tc = ctx.enter_context(tile.TileContext(nc))
        xp = ctx.enter_context(tc.tile_pool(name="xp", bufs=10))
        wp = ctx.enter_context(tc.tile_pool(name="wp", bufs=2))
        sp = ctx.enter_context(tc.tile_pool(name="sp", bufs=2))
        st = ctx.enter_context(tc.tile_pool(name="st", bufs=1))

        zero = st.tile([P, FREE], f32)
        nc.vector.memset(zero[:], 0.0)
        sig_bias = st.tile([P, 1], f32)
        nc.vector.memset(sig_bias[:], SIG_BIAS)

        wt_prev = None
        prev_tc = None
        spt = None
        g = 0  # current store group index
        goff = 0  # steps already swept into the current group tile
        for c, tcsz in enumerate(CHUNKS * reps):
            cc = c % len(CHUNKS)
            t0 = sum(CHUNKS[:cc])
            xt = xp.tile([P, TCMAX, FREE], f32, tag="x")
            # All loads ride the SP HWDGE ring (the GpSimd SWDGE path has
            # ~3.4 us software trigger latency and a ~12 us first-byte
            # delay); one HWDGE queue fans over all 16 DMA engines.
            nc.sync.dma_start(out=xt[:, :tcsz, :], in_=x_d[:, t0 : t0 + tcsz, :])

            wt = wp.tile([P, TCMAX, FREE], f32, tag="w")
            for j in range(tcsz):
                if c == 0 and j == 0:
                    w_in = zero[:]
                elif j == 0:
                    w_in = wt_prev[:, prev_tc - 1, :]
                else:
                    w_in = wt[:, j - 1, :]
                # w_t = select(w_{t-1} < TH, w_{t-1}, 0) * DECAY + x_t
                nc.vector._custom_dve(
                    lif_op,
                    out=wt[:, j, :],
                    in0=w_in,
                    in1=xt[:, j, :],
                    s0=TH,
                    s1=DECAY,
                )
            wt_prev = wt
            prev_tc = tcsz

            # spikes = u8(sigmoid(2^29*(w - 1.5))) — exact 0/1 indicator of
            # (w >= 1.5) up to the measure-zero w == 1.5 case — written into
            # this chunk's slice of the group store tile.
            if cc == GROUPS[g % len(GROUPS)][0]:
                spt = sp.tile([P, GMAX, FREE], u8, tag="s")
                goff = 0
            nc.scalar.activation(
                out=spt[:, goff : goff + tcsz, :].rearrange("p t f -> p (t f)"),
                in_=wt[:, :tcsz, :].rearrange("p t f -> p (t f)"),
                func=mybir.ActivationFunctionType.Sigmoid,
                bias=sig_bias[:],
                scale=SIG_SCALE,
            )
            goff += tcsz
            if cc == GROUPS[g % len(GROUPS)][1] - 1:
                ts = t0 + tcsz - goff
                nc.scalar.dma_start(
                    out=s_d[:, ts : ts + goff, :], in_=spt[:, :goff, :]
                )
                g += 1

    nc.compile()
    return nc


def _get_nc():
    if "nc" not in _CACHE:
        _CACHE["nc"] = _build_bass()
    return _CACHE["nc"]


def _shard_input(inputs: np.ndarray, i: int) -> np.ndarray:
    # [32, 200, 1024] -> [32, 200, 4, 256] -> [4, 32, 200, 256] -> [128, 200, 256]
    xi = inputs[i * BL : (i + 1) * BL]
    xi = xi.reshape(BL, T, NK, FREE).transpose(2, 0, 1, 3)
    return np.ascontiguousarray(xi).reshape(P128, T, FREE)


def _unshard_output(spk: np.ndarray) -> np.ndarray:
    # u8 [128, 200, 256] -> [4, 32, 200, 256] -> [32, 200, 4, 256]
    # -> [32, 200, 1024] f32 (u8 spikes are exact 0/1)
    s = spk.reshape(NK, BL, T, FREE).transpose(1, 2, 0, 3)
    return np.ascontiguousarray(s).reshape(BL, T, N).astype(np.float32)


def kernel(inputs: np.ndarray, trace: bool = False) -> np.ndarray:
    from concourse.bass_utils import run_bass_kernel_spmd

    inputs = np.ascontiguousarray(np.asarray(inputs, dtype=np.float32))
    assert inputs.shape == (B, T, N), inputs.shape

    nc = _get_nc()
    in_maps = [{"x": _shard_input(inputs, i)} for i in range(NCORES)]
    res = run_bass_kernel_spmd(
        nc, in_maps, core_ids=list(range(NCORES)), trace=trace
    )
    _CACHE["last_results"] = res
    out = np.concatenate(
        [_unshard_output(r["spk"]) for r in res.results], axis=0
    )
    return out
